# revision 1
# baseline (speedup 1.0000x reference)
"""Contrastive loss (CLIP-style, 2 views) on 8 Trainium2 NeuronCores.

Math: with Af/Bf the L2-normalized (V*N, D) view-major matrices,
  loss = mean_i [ logsumexp_{j != i}(Af@Bf.T / T)[i, :] - (Af@Bf.T)[i, p(i)]/T ]
where p(i) = (i + N) mod (V*N) is the other-view partner of row i.
The reference's mask/gather/sort is cosmetic: log_softmax is permutation
invariant, so only "drop the diagonal" and "read the partner column" matter.

Sharding: rows of Af are split across 8 cores (1024 rows each); every core
gets the full B (D-major) with its columns rotated by 1024*k so that the
diagonal of core k's slab lands at *static* local columns (row-chunk m ->
cols [128m, 128m+128) of column-group 0) and the partner diagonal at the
same offset of column-group 2.  This keeps the SPMD program identical on
all cores.  A's per-row 1/(|a|*T) is folded into the Exp activation scale,
so A itself is never normalized on-chip; B is normalized in place (square,
ones-matmul partition-reduce -> broadcast ss, sqrt, approx-reciprocal, mul).

Per core: 256-wide contraction split into 2 K-chunks; logits computed in
(128 x 2048) PSUM tiles (4 banks) with bf16 matmuls (1 row/cycle; operands
rounded to bf16 only after fp32 normalization), fused Exp+row-accumulate on
ACT, diagonal masked additively with -1e9, partner extracted with an
identity-mask multiply + row reduce.  Output is a (128, 1) per-partition
partial sum of (LSE_i - pos_i/T); the host adds the 8*128 partials and
divides by 8192.
"""

import os

import numpy as np

N = 4096
V = 2
D = 256
M = V * N            # 8192 rows/cols of the logits matrix
TEMP = 0.07
NCORES = 8
ROWS = M // NCORES   # 1024 rows per core
P = 128              # partitions
NM = ROWS // P       # 8 row-chunks per core
GW = 2048            # column-group width (one B DMA/normalize unit)
NG = M // GW         # 4 column groups
PSW = int(os.environ.get("KERNEL_PSW", "2048"))  # PSUM tile width
PBUFS = 4096 // PSW  # use all 8 PSUM banks: 2048 -> 2 bufs, 1024 -> 4
NSUB = GW // PSW     # PSUM tiles per column group
NST = M // PSW       # exp accumulator columns per row-chunk
KC = D // P          # 2 contraction chunks
NEG = -1.0e9         # additive mask for the diagonal
# bf16 default: fp8 DoubleRow halves PE time but the kernel is ACT-bound
# (measured identical wall time), so bf16's ~170x better accuracy is free
USE_FP8 = os.environ.get("KERNEL_FP8", "0") != "0"

_CACHE: dict = {}


def _build_nc():
    import concourse.bacc as bacc
    import concourse.bass as bass
    import concourse.mybir as mybir
    import concourse.tile as tile

    f32 = mybir.dt.float32
    bf16 = mybir.dt.bfloat16
    mmdt = mybir.dt.float8e4 if USE_FP8 else bf16
    mm_kwargs = (
        {"perf_mode": mybir.MatmulPerfMode.DoubleRow} if USE_FP8 else {})
    AX = mybir.AxisListType
    OP = mybir.AluOpType
    AF = mybir.ActivationFunctionType

    nc = bacc.Bacc("TRN2", target_bir_lowering=False, debug=False,
                   num_devices=NCORES)

    at_d = nc.dram_tensor("at", (D, ROWS), f32, kind="ExternalInput")
    arow_d = nc.dram_tensor("arow", (ROWS, D), f32, kind="ExternalInput")
    bt_d = nc.dram_tensor("bt", (D, M), f32, kind="ExternalInput")
    dmask_d = nc.dram_tensor("dmask", (P, P), f32, kind="ExternalInput")
    i128_d = nc.dram_tensor("i128", (P, P), f32, kind="ExternalInput")
    out_d = nc.dram_tensor("partials", (P, 1), f32, kind="ExternalOutput")

    with tile.TileContext(nc) as tc:
        with (
            tc.tile_pool(name="big", bufs=1) as big,
            tc.tile_pool(name="work", bufs=2) as work,
            tc.tile_pool(name="psum", bufs=2, space=bass.MemorySpace.PSUM) as pp,
        ):
            # --- persistent SBUF tensors -------------------------------
            at_s = big.tile((P, KC, ROWS), f32)     # A slab, D-major, fp32
            at_b = big.tile((P, KC, ROWS), mmdt)    # A slab (matmul lhsT)
            arow_s = big.tile((P, NM, D), f32)      # A slab, row-major
            bt_b = big.tile((P, KC, M), mmdt)       # normalized B (matmul rhs)
            dmask_s = big.tile((P, P), f32)
            i128_s = big.tile((P, P), f32)
            ones_s = big.tile((P, P), bf16)
            ssa_s = big.tile((P, NM), f32)          # sum(a^2) per slab row
            sqa_s = big.tile((P, NM), f32)
            sca_s = big.tile((P, NM), f32)          # 1/(|a|*T) per slab row
            acc_s = big.tile((P, NM * NST), f32)    # exp row-sums
            praw_s = big.tile((P, NM), f32)         # raw partner dots
            ssum_s = big.tile((P, NM), f32)
            lns_s = big.tile((P, NM), f32)
            lt_s = big.tile((P, NM), f32)
            outp_s = big.tile((P, 1), f32)

            # B group DMAs go first so group 0 lands as early as possible;
            # each dma_start is striped over all 16 DMA engines by the DGE
            btf_tiles = []
            for g in range(NG):
                gsl = slice(g * GW, (g + 1) * GW)
                btf = work.tile((P, KC, GW), f32, tag="btf", bufs=4)
                btf_tiles.append(btf)
                for kc in range(KC):
                    nc.sync.dma_start(
                        btf[:, kc, :],
                        bt_d.ap()[kc * P : (kc + 1) * P, gsl])
            nc.sync.dma_start(
                at_s[:], at_d.ap().rearrange("(k p) r -> p k r", p=P))
            nc.sync.dma_start(
                arow_s[:], arow_d.ap().rearrange("(t p) d -> p t d", p=P))
            nc.sync.dma_start(dmask_s[:], dmask_d.ap())
            nc.sync.dma_start(i128_s[:], i128_d.ap())
            nc.vector.memset(ones_s[:], 1.0)
            # off DVE's and ACT's critical paths (DVE gates the first
            # B-norm square, ACT the exps)
            nc.gpsimd.tensor_copy(at_b[:], at_s[:])

            # --- A row scales: 1 / (|a_i| * T) -------------------------
            # (tensor_tensor_reduce hard-faults the exec unit on this HW
            # path, so square and reduce are separate instructions)
            for m in range(NM):
                asq = work.tile((P, D), f32, tag="asq")
                nc.vector.tensor_mul(asq[:], arow_s[:, m, :], arow_s[:, m, :])
                nc.vector.reduce_sum(ssa_s[:, m : m + 1], asq[:], axis=AX.X)
            nc.scalar.sqrt(sqa_s[:], ssa_s[:])
            nc.vector.reciprocal_approx_fast(out=sca_s[:], in_=sqa_s[:])
            nc.vector.tensor_scalar_mul(sca_s[:], sca_s[:], 1.0 / TEMP)

            # --- phase 0: normalize all of B (keeps ACT tables stable:
            # all Sqrt here, all Exp later) ------------------------------
            for g in range(NG):
                btf = btf_tiles[g]
                for sub in range(NSUB):
                    ssl = slice(sub * PSW, (sub + 1) * PSW)
                    osl = slice(g * GW + sub * PSW, g * GW + (sub + 1) * PSW)
                    # B norms: ss broadcast over partitions via ones-matmul
                    ssb = pp.tile((P, PSW), f32, tag="ps", bufs=PBUFS)
                    for kc in range(KC):
                        bsq = work.tile((P, PSW), bf16, tag="bsq")
                        nc.vector.tensor_mul(bsq[:], btf[:, kc, ssl],
                                             btf[:, kc, ssl])
                        for c in range(PSW // 512):
                            csl = slice(c * 512, (c + 1) * 512)
                            nc.tensor.matmul(
                                ssb[:, csl],
                                ones_s[:],
                                bsq[:, csl],
                                start=(kc == 0), stop=(kc == KC - 1))
                    sqb = work.tile((P, PSW), f32, tag="sqb")
                    nc.scalar.sqrt(sqb[:], ssb[:])
                    invb = work.tile((P, PSW), f32, tag="invb")
                    nc.vector.reciprocal_approx_fast(out=invb[:], in_=sqb[:])
                    for kc in range(KC):
                        # normalize in fp32, rounding only on the write
                        nc.vector.tensor_mul(bt_b[:, kc, osl],
                                             btf[:, kc, ssl], invb[:])

            # --- phase 1: logits + exp row-sums ------------------------
            for g in range(NG):
                for m in range(NM):
                    dsub = (m * P) // PSW  # sub-tile holding the diagonal
                    for sub in range(NSUB):
                        lg = pp.tile((P, PSW), f32, tag="ps", bufs=PBUFS)
                        base = g * GW + sub * PSW
                        if USE_FP8:
                            # DoubleRow: both K-halves in one matmul via
                            # the 3D [128, 2, N] APs
                            for c in range(PSW // 512):
                                csl = slice(c * 512, (c + 1) * 512)
                                bsl = slice(base + c * 512,
                                            base + (c + 1) * 512)
                                nc.tensor.matmul(
                                    lg[:, csl],
                                    at_b[:, :, m * P : (m + 1) * P],
                                    bt_b[:, :, bsl],
                                    start=True, stop=True, **mm_kwargs)
                        else:
                            # kc outer: each A weight tile streams all banks
                            for kc in range(KC):
                                for c in range(PSW // 512):
                                    csl = slice(c * 512, (c + 1) * 512)
                                    bsl = slice(base + c * 512,
                                                base + (c + 1) * 512)
                                    nc.tensor.matmul(
                                        lg[:, csl],
                                        at_b[:, kc, m * P : (m + 1) * P],
                                        bt_b[:, kc, bsl],
                                        start=(kc == 0),
                                        stop=(kc == KC - 1),
                                        skip_group_check=True)
                        if sub == dsub:
                            msl = slice(m * P - dsub * PSW,
                                        m * P - dsub * PSW + P)
                            if g == 0:
                                # additive -1e9 on the diagonal -> exp == 0
                                nc.vector.tensor_add(lg[:, msl], lg[:, msl],
                                                     dmask_s[:])
                            if g == 2:
                                # partner (positive) dot on this diagonal
                                pscr = work.tile((P, P), f32, tag="pscr")
                                nc.vector.tensor_mul(pscr[:], lg[:, msl],
                                                     i128_s[:])
                                nc.vector.reduce_sum(
                                    praw_s[:, m : m + 1], pscr[:], axis=AX.X)
                        esc = work.tile((P, PSW), f32, tag="esc")
                        ai = m * NST + g * NSUB + sub
                        nc.scalar.activation(
                            esc[:], lg[:], AF.Exp,
                            bias=0.0, scale=sca_s[:, m : m + 1],
                            accum_out=acc_s[:, ai : ai + 1])

            # --- assembly: loss rows = ln(S) - praw * sca --------------
            for m in range(NM):
                nc.vector.reduce_sum(
                    ssum_s[:, m : m + 1], acc_s[:, m * NST : (m + 1) * NST],
                    axis=AX.X)
            nc.scalar.activation(lns_s[:], ssum_s[:], AF.Ln)
            nc.vector.tensor_mul(praw_s[:], praw_s[:], sca_s[:])
            nc.vector.tensor_sub(lt_s[:], lns_s[:], praw_s[:])
            nc.vector.reduce_sum(outp_s[:], lt_s[:], axis=AX.X)
            nc.sync.dma_start(out_d.ap(), outp_s[:])

    nc.compile()
    return nc


def get_nc():
    if "nc" not in _CACHE:
        _CACHE["nc"] = _build_nc()
    return _CACHE["nc"]


def make_in_maps(A: np.ndarray, B: np.ndarray) -> list[dict]:
    A = np.asarray(A, dtype=np.float32)
    B = np.asarray(B, dtype=np.float32)
    # view-major D-major matrices: X[d, v*N + n] = X_in[n, v, d]
    At = np.ascontiguousarray(A.transpose(2, 1, 0).reshape(D, M))
    Bt = np.ascontiguousarray(B.transpose(2, 1, 0).reshape(D, M))
    dmask = np.zeros((P, P), dtype=np.float32)
    np.fill_diagonal(dmask, NEG)
    i128 = np.eye(P, dtype=np.float32)
    in_maps = []
    for k in range(NCORES):
        at_k = np.ascontiguousarray(At[:, k * ROWS : (k + 1) * ROWS])
        arow_k = np.ascontiguousarray(at_k.T)
        # rotate columns so local col j holds global col (j + 1024k) % 8192
        bt_k = np.ascontiguousarray(np.roll(Bt, -ROWS * k, axis=1))
        in_maps.append({"at": at_k, "arow": arow_k, "bt": bt_k,
                        "dmask": dmask, "i128": i128})
    return in_maps


def kernel(A: np.ndarray, B: np.ndarray) -> np.ndarray:
    from concourse.bass_utils import run_bass_kernel_spmd

    in_maps = make_in_maps(A, B)
    nc = get_nc()
    trace = bool(int(os.environ.get("KERNEL_TRACE", "0")))
    res = run_bass_kernel_spmd(
        nc, in_maps, core_ids=list(range(NCORES)), trace=trace)
    total = 0.0
    for r in res.results:
        total += float(r["partials"].astype(np.float64).sum())
    if res.exec_time_ns is not None:
        print(f"[kernel] exec_time_ns={res.exec_time_ns}")
        _CACHE["exec_time_ns"] = res.exec_time_ns
    _CACHE["last_results"] = res
    return np.float32(total / M)

